# revision 1
# baseline (speedup 1.0000x reference)
"""DiscoNetFusion Trainium2 kernel (8 NeuronCores, SPMD).

Strategy
--------
Only ego agent i=0 of each scene contributes to the output, so per scene b we
need the L_b = record_len[b] neighbor warps nbr[b,0,j], the 4-layer 1x1-conv
attention head on z=[nbr;ego], a softmax over j, and the weighted feature sum
followed by a channel MLP.

Core k handles output rows [10k, 10k+10) of ALL scenes (8 cores x 10 rows =
80 rows).  Per core there are sum(record_len)=9 (scene, agent) units; each
unit is 1600 output pixels (padded to 1664 = 13 tiles of 128).

Bilinear warp = DMA gather (pixel-major dup-row fp16 source; one index
fetches the 2x2 tap patch: entries (y0,x0),(y0,x0+1), each entry holding
rows y0 and y0+1 of all 64 channels) + a lerp combine on the vector engine
with per-partition (=per-pixel) scalars, then a PE transpose back to
channel-major for the conv stack.  Softmax/attention is computed with exp on
the scalar engine, j-reduction + partition-broadcasts via tiny PE matmuls.

Host side (numpy) only prepares gather indices / lerp weights / warped-mask
maps (data-independent of x) and reassembles the 8 row-blocks.
"""

import dataclasses
import os

import numpy as np

import concourse.bacc as bacc
import concourse.mybir as mybir
from concourse.bass_utils import run_bass_kernel_spmd
from concourse.tile import TileContext

F32 = mybir.dt.float32
F16 = mybir.dt.float16
I16 = mybir.dt.int16
Alu = mybir.AluOpType
Act = mybir.ActivationFunctionType

C = 64
H = 80
W = 160
B = 3
L = 4
EPS = 1e-5
NCORES = 8
R = H // NCORES            # output rows per core
PX = R * W                 # 1600 real pixels
NT = 13                    # px tiles of 128
PXP = NT * 128             # 1664 padded pixels
NENT = H * W               # gather source entries per agent
CHUNKS = [(0, 512), (512, 512), (1024, 512), (1536, 128)]
HCHUNKS = [(0, 832, [(0, 512), (512, 320)]), (832, 832, [(0, 512), (512, 320)])]
OUT_CHUNKS = [(0, 512), (512, 512), (1024, 512), (1536, 64)]


def _wrap_idx(idx_flat):
    """[N] -> [128, N//16] wrapped-in-16-partitions, replicated to 8 groups."""
    n = idx_flat.shape[0]
    w = idx_flat.reshape(n // 16, 16).T  # [16, N//16]
    return np.tile(w, (8, 1)).astype(np.int16)


def _host_warp_prep(theta, h0):
    """Per-(unit) gather indices + lerp scalars for output rows [h0,h0+R).

    Returns idx[PXP] int32 (entry units), fx, c0, c1 [PXP] f32 and
    (y0,x0,fy,fx,scale) pieces needed for the mask warp.
    """
    ys = np.linspace(-1.0, 1.0, H, dtype=np.float32)[h0 : h0 + R]
    xs = np.linspace(-1.0, 1.0, W, dtype=np.float32)
    gx, gy = np.meshgrid(xs, ys)  # [R, W]
    sx = theta[0, 0] * gx + theta[0, 1] * gy + theta[0, 2]
    sy = theta[1, 0] * gx + theta[1, 1] * gy + theta[1, 2]
    px = (sx + 1.0) * (W - 1) / 2.0
    py = (sy + 1.0) * (H - 1) / 2.0
    x0 = np.floor(px).astype(np.int64)
    y0 = np.floor(py).astype(np.int64)
    fx = (px - x0).astype(np.float32)
    fy = (py - y0).astype(np.float32)

    scale = np.ones_like(fx)
    # x handling
    x0c = np.clip(x0, 0, W - 1)
    fxp = fx.copy()
    m = x0 == W - 1          # x1 out of bounds -> drop B/D taps
    fxp[m] = 0.0
    scale[m] *= 1.0 - fx[m]
    m = x0 == -1             # x0 out of bounds -> entry at x=0 is the B tap
    x0c[m] = 0
    fxp[m] = 0.0
    scale[m] *= fx[m]
    m = (x0 < -1) | (x0 > W - 1)
    x0c[m] = 0
    fxp[m] = 0.0
    scale[m] = 0.0
    # y handling (entry [y0] holds rows y0,y0+1; row 80 half is zeros)
    y0c = np.clip(y0, 0, H - 1)
    fyp = fy.copy()
    m = y0 == -1             # row0 is the F tap
    y0c[m] = 0
    fyp[m] = 0.0
    scale[m] *= fy[m]
    m = (y0 < -1) | (y0 > H - 1)
    y0c[m] = 0
    fyp[m] = 0.0
    scale[m] = 0.0

    idx = (y0c * W + x0c).reshape(-1)
    c0 = (scale * (1.0 - fyp)).reshape(-1)
    c1 = (scale * fyp).reshape(-1)
    fxp = fxp.reshape(-1)

    pad = PXP - PX
    idx = np.concatenate([idx, np.zeros(pad, np.int64)])
    fxp = np.concatenate([fxp, np.zeros(pad, np.float32)])
    c0 = np.concatenate([c0, np.zeros(pad, np.float32)])
    c1 = np.concatenate([c1, np.zeros(pad, np.float32)])
    return idx, fxp, c0, c1


def _host_warp_mask(mask_bj, theta, h0):
    """Bilinear warp of one [H,W] mask (zero padding) for rows [h0,h0+R)."""
    ys = np.linspace(-1.0, 1.0, H, dtype=np.float32)[h0 : h0 + R]
    xs = np.linspace(-1.0, 1.0, W, dtype=np.float32)
    gx, gy = np.meshgrid(xs, ys)
    sx = theta[0, 0] * gx + theta[0, 1] * gy + theta[0, 2]
    sy = theta[1, 0] * gx + theta[1, 1] * gy + theta[1, 2]
    px = (sx + 1.0) * (W - 1) / 2.0
    py = (sy + 1.0) * (H - 1) / 2.0
    x0 = np.floor(px).astype(np.int64)
    y0 = np.floor(py).astype(np.int64)
    wx = (px - x0).astype(np.float32)
    wy = (py - y0).astype(np.float32)

    def gat(xi, yi):
        inb = ((xi >= 0) & (xi < W) & (yi >= 0) & (yi < H)).astype(np.float32)
        v = mask_bj[np.clip(yi, 0, H - 1), np.clip(xi, 0, W - 1)]
        return v * inb

    out = (
        gat(x0, y0) * (1 - wx) * (1 - wy)
        + gat(x0 + 1, y0) * wx * (1 - wy)
        + gat(x0, y0 + 1) * (1 - wx) * wy
        + gat(x0 + 1, y0 + 1) * wx * wy
    )
    return out.reshape(-1)  # [PX]


class _StageDone(Exception):
    pass


def _build_program(nagents, scene_of, src_names):
    """Build the SPMD Bass program (identical for all cores)."""
    nc = bacc.Bacc("TRN2", target_bir_lowering=False, num_devices=NCORES)

    srcs = [
        nc.dram_tensor(nm, [NENT + 1, 2 * C], F16, kind="ExternalInput")
        for nm in src_names
    ]
    idx_all = nc.dram_tensor("idx_all", [128, nagents * (PXP // 16)], I16,
                             kind="ExternalInput")
    scal_all = nc.dram_tensor("scal_all", [128, nagents * 8 * NT], F16,
                              kind="ExternalInput")
    ego_all = nc.dram_tensor("ego_all", [C, B * PXP], F16, kind="ExternalInput")
    cm_all = nc.dram_tensor("cm_all", [nagents, 2 * PXP], F16,
                            kind="ExternalInput")
    w1 = nc.dram_tensor("w1", [2 * C, 2 * C], F16, kind="ExternalInput")
    w2 = nc.dram_tensor("w2", [2 * C, 32], F16, kind="ExternalInput")
    w3 = nc.dram_tensor("w3", [32, 32], F16, kind="ExternalInput")
    w4 = nc.dram_tensor("w4", [8, 32], F16, kind="ExternalInput")
    mlpw = nc.dram_tensor("mlpw", [C, C], F16, kind="ExternalInput")
    # per-partition scale/bias vectors: [128, 6] f32
    #   col0: a1, col1: b1, col2: a2, col3: b2, col4: a3, col5: b3
    sb = nc.dram_tensor("sb", [128, 6], F32, kind="ExternalInput")
    cb4b = nc.dram_tensor("cb4b", [65, 1], F32, kind="ExternalInput")
    sb2 = nc.dram_tensor("sb2", [96, 1], F32, kind="ExternalInput")
    sb3 = nc.dram_tensor("sb3", [72, 1], F32, kind="ExternalInput")
    mlpb = nc.dram_tensor("mlpb", [C, 1], F32, kind="ExternalInput")
    ident = nc.dram_tensor("ident", [128, 128], F16, kind="ExternalInput")
    ones64 = nc.dram_tensor("ones64", [1, C], F16, kind="ExternalInput")
    ind_js = nc.dram_tensor("ind_js", [nagents, B], F16, kind="ExternalInput")
    ind_sj = nc.dram_tensor("ind_sj", [B, nagents], F16, kind="ExternalInput")
    npair = (nagents + 1) // 2
    pairsel = nc.dram_tensor("pairsel", [nagents, npair * 128], F16,
                             kind="ExternalInput")
    out = nc.dram_tensor("out", [B * C, PX], F32, kind="ExternalOutput")
    debug = bool(os.environ.get("KERNEL_DEBUG"))
    if debug:
        dbg_s = nc.dram_tensor("dbg_s", [16, PXP], F16, kind="ExternalOutput")
        dbg_z = nc.dram_tensor("dbg_z", [128, PXP], F16, kind="ExternalOutput")
        dbg_alp = nc.dram_tensor("dbg_alp", [16, PXP], F16, kind="ExternalOutput")
        dbg_u = nc.dram_tensor("dbg_u", [C, PXP], F16, kind="ExternalOutput")
        dbg_h1 = nc.dram_tensor("dbg_h1", [128, 512], F16, kind="ExternalOutput")
        dbg_h2 = nc.dram_tensor("dbg_h2", [32, 512], F16, kind="ExternalOutput")
        dbg_h3 = nc.dram_tensor("dbg_h3", [8, 512], F16, kind="ExternalOutput")

    stage = int(os.environ.get("KERNEL_STAGE", "3"))
    with TileContext(nc) as tc:
        with (
            tc.tile_pool(name="const", bufs=1) as cpool,
            tc.tile_pool(name="zs", bufs=1) as zpool,
            tc.tile_pool(name="work", bufs=2) as wpool,
            tc.tile_pool(name="att", bufs=1) as apool,
            tc.tile_pool(name="pmm", bufs=1, space="PSUM") as pmm,
            tc.tile_pool(name="ptr", bufs=2, space="PSUM") as ptr,
        ):
            # ---- constants ----
            idx_t = cpool.tile([128, nagents * (PXP // 16)], I16)
            nc.sync.dma_start(idx_t[:], idx_all[:, :])
            scal_t = cpool.tile([128, nagents * 8 * NT], F16)
            nc.sync.dma_start(scal_t[:], scal_all[:, :])
            ego_t = cpool.tile([C, B * PXP], F16)
            nc.sync.dma_start(ego_t[:], ego_all[:, :])
            cm_t = cpool.tile([nagents, 2 * PXP], F16)
            nc.sync.dma_start(cm_t[:], cm_all[:, :])
            w1_t = cpool.tile([2 * C, 2 * C], F16)
            nc.sync.dma_start(w1_t[:], w1[:, :])
            w2_t = cpool.tile([2 * C, 32], F16)
            nc.sync.dma_start(w2_t[:], w2[:, :])
            w3_t = cpool.tile([128, 32], F16)
            w4_t = cpool.tile([128, 32], F16)
            for q in range(3):
                nc.sync.dma_start(w3_t[32 * q : 32 * q + 32, :], w3[:, :])
                nc.sync.dma_start(w4_t[32 * q : 32 * q + 8, :], w4[:, :])
            mlpw_t = cpool.tile([C, C], F16)
            nc.sync.dma_start(mlpw_t[:], mlpw[:, :])
            sb_t = cpool.tile([128, 6], F32)
            nc.sync.dma_start(sb_t[:], sb[:, :])
            cb4_t = cpool.tile([65, 1], F32)
            nc.sync.dma_start(cb4_t[:], cb4b[:, :])
            mlpb_t = cpool.tile([C, 1], F32)
            nc.sync.dma_start(mlpb_t[:], mlpb[:, :])
            id_t = cpool.tile([128, 128], F16)
            nc.sync.dma_start(id_t[:], ident[:, :])
            ones_t = cpool.tile([1, C], F16)
            nc.sync.dma_start(ones_t[:], ones64[:, :])
            indjs_t = cpool.tile([nagents, B], F16)
            nc.sync.dma_start(indjs_t[:], ind_js[:, :])
            indsj_t = cpool.tile([B, nagents], F16)
            nc.sync.dma_start(indsj_t[:], ind_sj[:, :])
            pairsel_t = cpool.tile([nagents, npair * 128], F16)
            nc.sync.dma_start(pairsel_t[:], pairsel[:, :])

            z_all = [zpool.tile([128, PXP], F16, name=f"z{j}", tag=f"z{j}")
                     for j in range(nagents)]
            h1_all = [None, None, None]
            sb2_t = cpool.tile([96, 1], F32)
            nc.sync.dma_start(sb2_t[:], sb2[:, :])
            sb3_t = cpool.tile([72, 1], F32)
            nc.sync.dma_start(sb3_t[:], sb3[:, :])
            s_all = apool.tile([nagents, PXP], F16)

            for j in range(nagents):
                b = scene_of[j]
                # ---- gather 2x2 taps, pixel-major ----
                g_t = wpool.tile([128, NT, 4 * C], F16, tag="g", bufs=3)
                src_flat = srcs[j][:, :].rearrange("a b -> (a b)")
                src_win = dataclasses.replace(
                    src_flat, ap=[[2 * C, NENT], [1, 4 * C]]
                )
                for (gt0, gtn) in ((0, 7), (7, 6)):
                    nc.gpsimd.dma_gather(
                        g_t[:, gt0 : gt0 + gtn, :],
                        src_win,
                        idx_t[:, j * (PXP // 16) + gt0 * 8 :
                              j * (PXP // 16) + (gt0 + gtn) * 8],
                        num_idxs=gtn * 128,
                        num_idxs_reg=gtn * 128,
                        elem_size=4 * C,
                        elem_step=2 * C,
                        single_packet=False,
                    )
                # ---- bilinear combine: nbr = w00*A+w01*B + w10*C+w11*D ----
                # weights live packed [128, NT, 4] (w00,w10,w01,w11); read with
                # free-step-0 APs to broadcast each weight over 64 channels.
                t1_t = wpool.tile([128, NT, 2 * C], F16, tag="t1", bufs=3)
                t2_t = wpool.tile([128, NT, 2 * C], F16, tag="t2", bufs=3)
                nbr_t = wpool.tile([128, NT, C], F16, tag="nbr", bufs=3)
                # weights stored duplicated in pairs: col 8t+2q+d = w_q[tile t]
                # one mult per tap block keeps free dims at 3 (walrus limit)
                # while the packed [1,2] last dim preserves the DVE 2x mode
                wq = scal_t[:, j * 8 * NT : (j + 1) * 8 * NT]
                for q, dst in ((0, t1_t[:, :, 0:C]), (1, t1_t[:, :, C : 2 * C]),
                               (2, t2_t[:, :, 0:C]), (3, t2_t[:, :, C : 2 * C])):
                    w_ap = dataclasses.replace(
                        wq, offset=wq.offset + 2 * q,
                        ap=[list(wq.ap[0]), [8, NT], [0, C // 2], [1, 2]])
                    src = g_t[:, :, q * C : (q + 1) * C]
                    nc.vector.tensor_tensor(
                        dst.rearrange("p a (c d) -> p a c d", d=2),
                        src.rearrange("p a (c d) -> p a c d", d=2),
                        w_ap, Alu.mult)
                nc.vector.tensor_tensor(t1_t[:, :, 0:C], t1_t[:, :, 0:C],
                                        t2_t[:, :, 0:C], Alu.add)
                nc.vector.tensor_tensor(t1_t[:, :, C : 2 * C],
                                        t1_t[:, :, C : 2 * C],
                                        t2_t[:, :, C : 2 * C], Alu.add)
                nc.vector.tensor_tensor(
                    nbr_t[:], t1_t[:, :, 0:C], t1_t[:, :, C : 2 * C], Alu.add)
                # ---- transpose px-major -> channel-major into z ----
                # 4 transposes land at column offsets of one [64, 512] psum
                # bank; a single evac moves all 4 (alternating ACT / DVE)
                z_t = z_all[j]
                for t0 in range(0, NT, 4):
                    tn = min(4, NT - t0)
                    tr_ps = ptr.tile([C, 512], F16, tag="tr")
                    for t in range(t0, t0 + tn):
                        nc.tensor.transpose(
                            tr_ps[:, 128 * (t - t0) : 128 * (t - t0 + 1)],
                            nbr_t[:, t, :], id_t[:])
                    dst = z_t[0:C, 128 * t0 : 128 * (t0 + tn)]
                    nc.scalar.activation(dst, tr_ps[:, 0 : 128 * tn],
                                         Act.Copy)
                # ego half
                nc.sync.dma_start(
                    z_t[C : 2 * C, :], ego_t[:, b * PXP : (b + 1) * PXP]
                )
                # ---- conv1 + h1 (per agent; trio stages run below) ----
                if stage < 2:
                    continue
                h1_j = wpool.tile([128, PXP], F16, name=f"h1_{j}", tag=f"h1_{j % 3}")
                h1_all[j % 3] = h1_j
                for (o, n, mms) in HCHUNKS:
                    p1 = pmm.tile([128, 832], F32, tag="p1", bufs=1)
                    for (mo, mn) in mms:
                        nc.tensor.matmul(p1[:, mo : mo + mn], w1_t[:],
                                         z_t[:, o + mo : o + mo + mn],
                                         start=True, stop=True)
                    nc.scalar.activation(h1_j[:, o : o + n], p1[:, 0:n],
                                         Act.Relu, bias=sb_t[:, 1:2], scale=1.0)

                # ---- conv2..4 for a completed trio of agents ----
                if j % 3 == 2 or j == nagents - 1:
                    trio = [jj for jj in (j - j % 3 + q for q in range(3))
                            if jj <= j]
                    hs2 = wpool.tile([96, PXP], F16, tag="hs2")
                    hs3 = wpool.tile([72, PXP], F16, tag="hs3")
                    srow = wpool.tile([65, PXP], F16, tag="srow")
                    for (o, n, mms) in HCHUNKS:
                        sl = slice(o, o + n)
                        ph2 = pmm.tile([96, 832], F32, tag="p34", bufs=2)
                        for q, jj in enumerate(trio):
                            for (mo, mn) in mms:
                                nc.tensor.matmul(
                                    ph2[32 * q : 32 * q + 32, mo : mo + mn],
                                    w2_t[:],
                                    h1_all[q][:, o + mo : o + mo + mn],
                                    start=True, stop=True)
                        nc.scalar.activation(hs2[0 : 32 * len(trio), sl],
                                             ph2[0 : 32 * len(trio), 0:n],
                                             Act.Relu,
                                             bias=sb2_t[0 : 32 * len(trio), 0:1],
                                             scale=1.0)
                        p34 = pmm.tile([96, 832], F32, tag="p34", bufs=2)
                        for q, jj in enumerate(trio):
                            for (mo, mn) in mms:
                                nc.tensor.matmul(
                                    p34[32 * q : 32 * q + 32, mo : mo + mn],
                                    w3_t[32 * q : 32 * q + 32, :],
                                    hs2[32 * q : 32 * q + 32,
                                        o + mo : o + mo + mn],
                                    start=True, stop=True)
                        nc.scalar.activation(
                            hs3[0 : 32 * (len(trio) - 1) + 8, sl],
                            p34[0 : 32 * (len(trio) - 1) + 8, 0:n], Act.Relu,
                            bias=sb3_t[0 : 32 * (len(trio) - 1) + 8, 0:1],
                            scale=1.0)
                        p4 = pmm.tile([96, 832], F32, tag="p34", bufs=2)
                        for q, jj in enumerate(trio):
                            for (mo, mn) in mms:
                                nc.tensor.matmul(
                                    p4[32 * q : 32 * q + 32, mo : mo + mn],
                                    w4_t[32 * q : 32 * q + 8, :],
                                    hs3[32 * q : 32 * q + 8,
                                        o + mo : o + mo + mn],
                                    start=True, stop=True)
                        nc.scalar.activation(srow[0 : 32 * (len(trio) - 1) + 1, sl],
                                             p4[0 : 32 * (len(trio) - 1) + 1, 0:n],
                                             Act.Relu,
                                             bias=cb4_t[0 : 32 * (len(trio) - 1) + 1, 0:1],
                                             scale=1.0)
                    for q, jj in enumerate(trio):
                        nc.sync.dma_start(s_all[jj : jj + 1, :],
                                          srow[32 * q : 32 * q + 1, :])

            # ---- softmax over j (unnormalized exp; NEG-masked via cm==0) ----
            if stage < 3:
                if debug:
                    nc.sync.dma_start(dbg_z[:, :], z_all[0][:])
                    if stage >= 2:
                        nc.sync.dma_start(dbg_s[0:nagents, :], s_all[:])
                do_attention = False
            else:
                do_attention = True
            if do_attention:
                e_t = apool.tile([nagents, PXP], F16)
                nc.scalar.activation(e_t[:], s_all[:], Act.Exp)
                ep_t = apool.tile([nagents, PXP], F16)   # e * (cm != 0)
                nc.vector.tensor_tensor(ep_t[:], e_t[:], cm_t[:, PXP : 2 * PXP],
                                        Alu.mult)
                al_t = apool.tile([nagents, PXP], F16)   # alpha = e * cm
                nc.vector.tensor_tensor(al_t[:], e_t[:], cm_t[:, 0:PXP], Alu.mult)
                # den per scene + reciprocal + broadcast back to agents
                rec_t = apool.tile([B, PXP], F16)
                alp_t = apool.tile([nagents, PXP], F16)  # alpha / den
                for (o, n) in CHUNKS:
                    sl = slice(o, o + n)
                    dps = pmm.tile([B, 512], F32, tag="p34", bufs=2)
                    nc.tensor.matmul(dps[:, 0:n], indjs_t[:], ep_t[:, sl],
                                     start=True, stop=True)
                    with nc.allow_low_precision(reason="den>=1, fp16 rec ok"):
                        nc.vector.reciprocal(rec_t[:, sl], dps[:, 0:n])
                    rps = pmm.tile([nagents, 512], F32, tag="p34", bufs=2)
                    nc.tensor.matmul(rps[:, 0:n], indsj_t[:], rec_t[:, sl],
                                     start=True, stop=True)
                    nc.vector.tensor_tensor(alp_t[:, sl], al_t[:, sl], rps[:, 0:n],
                                            Alu.mult)

                # ---- weighted sum over agents, per scene ----
                u_all = [apool.tile([C, PXP], F16, name=f"u{b}", tag=f"u{b}")
                         for b in range(B)]
                first = {(b, o): True for b in range(B) for (o, n, _) in HCHUNKS}
                seen = set()
                for p in range(npair):
                    pj = [j for j in (2 * p, 2 * p + 1) if j < nagents]
                    for (o, n, mms) in HCHUNKS:
                        sl = slice(o, o + n)
                        abps = pmm.tile([128, 832], F32, tag="p1", bufs=1)
                        for (mo, mn) in mms:
                            nc.tensor.matmul(abps[:, mo : mo + mn],
                                             pairsel_t[:, 128 * p : 128 * (p + 1)],
                                             alp_t[:, o + mo : o + mo + mn],
                                             start=True, stop=True)
                        for ii, j in enumerate(pj):
                            b = scene_of[j]
                            half = abps[64 * ii : 64 * (ii + 1), 0:n]
                            if (j, o) not in seen:
                                seen.add((j, o))
                                if first[(b, o)]:
                                    first[(b, o)] = False
                                    nc.vector.tensor_tensor(
                                        u_all[b][:, sl], z_all[j][0:C, sl],
                                        half, Alu.mult,
                                    )
                                    continue
                            pr = wpool.tile([C, 832], F16, tag="pr")
                            nc.vector.tensor_tensor(
                                pr[:, 0:n], z_all[j][0:C, sl], half, Alu.mult
                            )
                            nc.vector.tensor_tensor(
                                u_all[b][:, sl], u_all[b][:, sl], pr[:, 0:n],
                                Alu.add,
                            )

                if debug:
                    nc.sync.dma_start(dbg_s[0:nagents, :], s_all[:])
                    nc.sync.dma_start(dbg_z[:, :], z_all[0][:])
                    nc.sync.dma_start(dbg_alp[0:nagents, :], alp_t[:])
                    nc.sync.dma_start(dbg_u[:, :], u_all[0][:])
                # ---- MLP + bias, write out ----
                for b in range(B):
                    for (o, n, mms) in [(0, 832, [(0, 512), (512, 320)]),
                                        (832, 768, [(0, 512), (512, 256)])]:
                        mps = pmm.tile([C, 832], F32, tag="p34", bufs=2)
                        for (mo, mn) in mms:
                            nc.tensor.matmul(mps[:, mo : mo + mn], mlpw_t[:],
                                             u_all[b][:, o + mo : o + mo + mn],
                                             start=True, stop=True)
                        ob = wpool.tile([C, 832], F32, tag="ob")
                        nc.scalar.activation(ob[:, 0:n], mps[:, 0:n],
                                             Act.Identity, bias=mlpb_t[:, 0:1],
                                             scale=1.0)
                        nc.sync.dma_start(out[b * C : (b + 1) * C, o : o + n],
                                          ob[:, 0:n])

    nc.compile()
    return nc


_PROG_CACHE = {}
_LAST_RES = None


def kernel(**inputs):
    x = np.asarray(inputs["x"], np.float32)
    mask = np.asarray(inputs["mask"], np.float32)
    record_len = np.asarray(inputs["record_len"])
    ptm = np.asarray(inputs["pairwise_t_matrix"], np.float32)
    rec = [int(v) for v in record_len]
    agents = [(b, j) for b in range(B) for j in range(rec[b])]
    nagents = len(agents)
    scene_of = [b for (b, j) in agents]

    # ---- regroup x into per-scene node features ----
    node = np.zeros((B, L, C, H, W), np.float32)
    idx0 = 0
    for b, n in enumerate(rec):
        node[b, :n] = x[idx0 : idx0 + n]
        idx0 += n

    # ---- gather sources: dup-row pixel-major fp16 ----
    src_names = [f"src{a}" for a in range(nagents)]
    src_arrs = {}
    for a, (b, j) in enumerate(agents):
        feat = node[b, j]  # [C, H, W]
        ent = np.zeros((H + 1, W, 2 * C), np.float16)
        pm = feat.transpose(1, 2, 0).astype(np.float16)  # [H, W, C]
        ent[:H, :, :C] = pm
        ent[:H - 1, :, C:] = pm[1:]
        # row H-1 second half stays zero (virtual row 80 = 0)
        arr = np.zeros((NENT + 1, 2 * C), np.float16)
        arr[:NENT] = ent[:H].reshape(NENT, 2 * C)
        src_arrs[src_names[a]] = arr

    # ---- per-core index/scalar/mask/ego prep ----
    per_core = []
    for k in range(NCORES):
        h0 = k * R
        idx_cols = np.zeros((128, nagents * (PXP // 16)), np.int16)
        scal_cols = np.zeros((128, nagents * 8 * NT), np.float16)
        cm_arr = np.zeros((nagents, 2 * PXP), np.float16)
        ego_arr = np.zeros((C, B * PXP), np.float16)
        for b in range(B):
            ego = node[b, 0][:, h0 : h0 + R, :].reshape(C, PX)
            ego_arr[:, b * PXP : b * PXP + PX] = ego.astype(np.float16)
        for a, (b, j) in enumerate(agents):
            theta = ptm[b, j, 0]  # theta[b, i=0, j] = ptm[b, j, 0]
            idx, fxp, c0, c1 = _host_warp_prep(theta, h0)
            idx_cols[:, a * (PXP // 16) : (a + 1) * (PXP // 16)] = _wrap_idx(idx)
            w00 = (c0 * (1.0 - fxp)).astype(np.float16)
            w10 = (c1 * (1.0 - fxp)).astype(np.float16)
            w01 = (c0 * fxp).astype(np.float16)
            w11 = (c1 * fxp).astype(np.float16)
            sc = scal_cols[:, a * 8 * NT : (a + 1) * 8 * NT]
            for t in range(NT):
                pxs = slice(128 * t, 128 * (t + 1))
                for q, wv in enumerate((w00, w10, w01, w11)):
                    sc[:, 8 * t + 2 * q] = wv[pxs]
                    sc[:, 8 * t + 2 * q + 1] = wv[pxs]
            wm = _host_warp_mask(mask[b, j], theta, h0)
            cm_arr[a, :PX] = wm.astype(np.float16)
            cm_arr[a, PXP : PXP + PX] = (wm != 0).astype(np.float16)
            cm_arr[a, PXP + PX :] = 1.0
        per_core.append((idx_cols, scal_cols, cm_arr, ego_arr))

    # ---- shared small tensors ----
    def gf(n):
        return np.asarray(inputs[n], np.float32)

    sb = np.zeros((128, 6), np.float32)
    sb2v = np.zeros((96, 1), np.float32)
    sb3v = np.zeros((72, 1), np.float32)
    a1 = gf("g1") / np.sqrt(gf("rv1") + EPS)
    sb[:, 1] = gf("be1") + (gf("cb1") - gf("rm1")) * a1
    a2 = gf("g2") / np.sqrt(gf("rv2") + EPS)
    b2f = gf("be2") + (gf("cb2") - gf("rm2")) * a2
    a3 = gf("g3") / np.sqrt(gf("rv3") + EPS)
    b3f = gf("be3") + (gf("cb3") - gf("rm3")) * a3
    for q in range(3):
        sb2v[32 * q : 32 * q + 32, 0] = b2f
        sb3v[32 * q : 32 * q + 8, 0] = b3f

    ind_js = np.zeros((nagents, B), np.float16)
    for a, bb in enumerate(scene_of):
        ind_js[a, bb] = 1.0
    npair = (nagents + 1) // 2
    psel = np.zeros((nagents, npair * 128), np.float16)
    for p in range(npair):
        psel[2 * p, 128 * p : 128 * p + 64] = 1.0
        if 2 * p + 1 < nagents:
            psel[2 * p + 1, 128 * p + 64 : 128 * (p + 1)] = 1.0
    shared = {
        "idx_all": None,  # per core
        "pairsel": psel,
        "w1": (gf("w1") * a1[None, :]).astype(np.float16),
        "w2": (gf("w2") * a2[None, :]).astype(np.float16),
        "w3": np.pad((gf("w3") * a3[None, :]).astype(np.float16),
                     ((0, 0), (0, 24))),
        "w4": np.pad(gf("w4").astype(np.float16), ((0, 0), (0, 31))),
        "mlpw": gf("mlp_w").astype(np.float16),
        "sb": sb,
        "sb2": sb2v,
        "sb3": sb3v,
        "cb4b": np.full((65, 1), gf("cb4")[0], np.float32),
        "mlpb": gf("mlp_b").reshape(C, 1),
        "ident": np.eye(128, dtype=np.float16),
        "ones64": np.ones((1, C), np.float16),
        "ind_js": ind_js,
        "ind_sj": ind_js.T.copy(),
    }
    shared.update(src_arrs)
    del shared["idx_all"]

    key = (nagents, tuple(scene_of))
    if key not in _PROG_CACHE:
        _PROG_CACHE[key] = _build_program(nagents, scene_of, src_names)
    nc = _PROG_CACHE[key]

    in_maps = []
    for k in range(NCORES):
        idx_cols, scal_cols, cm_arr, ego_arr = per_core[k]
        m = dict(shared)
        m["idx_all"] = idx_cols
        m["scal_all"] = scal_cols
        m["cm_all"] = cm_arr
        m["ego_all"] = ego_arr
        in_maps.append(m)

    trace = bool(os.environ.get("KERNEL_TRACE"))
    res = run_bass_kernel_spmd(nc, in_maps, core_ids=list(range(NCORES)),
                               trace=trace)
    global _LAST_RES
    _LAST_RES = res

    out = np.zeros((B, C, H, W), np.float32)
    for k in range(NCORES):
        o = res.results[k]["out"]  # [B*C, PX]
        out[:, :, k * R : (k + 1) * R, :] = o.reshape(B, C, R, W)
    return out

